# revision 15
# baseline (speedup 1.0000x reference)
"""Trainium2 Bass kernel for 16-head MultiHeadAttention (B=2, S=2048, D=1024).

Sharding: 8 cores = 2 (batch) x 4 (head groups of 4 heads).
Each core gets a col-shard of Wq/Wk/Wv ([1024,256]) + row-shard of Wo
([256,1024]) and emits ONE full [2048,1024] bf16 partial (K=256 PSUM
accumulation over the two head-pairs); the host sums 4 partials per batch.

All SBUF tensors are bf16 (PSUM accumulation stays f32); inputs arrive
bf16 from the host, halving DMA. Pipeline error ~6e-3 (gate 2e-2).

Pipeline (v3): phase B starts at ~18us because ALL projection work is
chunked into ~1MB-gated filler units that stream behind the first
score matmuls:
 - K/Q projections are chunked by 512-column groups (kproj/qproj units);
   only kproj(0)+qproj(0) (2.5MB of DMA) gate the first scores.
 - V is projected directly in [j, dg] layout (x-tile as the stationary
   operand -- no PE transposes); quarter i needs only xv's column-slice
   cc=i (1MB), so the quarters stream as slot-1 fillers.
 - 8 slots of (head-pair p, q-chunk c): score MM pairs (row-packed via
   tile_position, measured concurrent) interleave 1:1 with the previous
   slot's AV matmuls at j-tile granularity, paced by the ACT engine's
   exp (~1.09us per j-tile, the true bottleneck at ~139us).
 - AV rowsums via the ones-column of V_aug (M=65); the reciprocal runs
   128-wide after a [1,512]->[4,128] DMA fold, off the PE critical path
   (PSUM av banks are evacuated by rs/st copies immediately).
 - PSUM: sc 2x[128,2,512] (4 banks) + av 2 + filler 2 = 8 banks exactly.
"""

import sys

import numpy as np

if "/opt/trn_rl_repo" not in sys.path:
    sys.path.insert(0, "/opt/trn_rl_repo")

import ml_dtypes

import concourse.bacc as bacc
import concourse.mybir as mybir
import concourse.tile as tile

F32 = mybir.dt.float32
BF16 = mybir.dt.bfloat16
NPBF = ml_dtypes.bfloat16

B, S, D, H = 2, 2048, 1024, 16
DK = D // H          # 64
HL = 4               # heads per core
DG = HL * DK         # 256
SCALE = 0.125        # 1/sqrt(DK)

ET = D // 128        # 8 e-tiles (contraction tiles for projections)
JT = S // 128        # 16 j-tiles (key positions)
QC = 4               # q-chunks of 512 per head-pair
EXP = mybir.ActivationFunctionType.Exp
MULT = mybir.AluOpType.mult


def _build_nc():
    nc = bacc.Bacc("TRN2", target_bir_lowering=False, debug=False)

    xq = nc.dram_tensor("xq", [D, S], BF16, kind="ExternalInput").ap()
    xk = nc.dram_tensor("xk", [D, S], BF16, kind="ExternalInput").ap()
    xv = nc.dram_tensor("xv", [D, S], BF16, kind="ExternalInput").ap()
    wq = nc.dram_tensor("wq", [D, DG], BF16, kind="ExternalInput").ap()
    wk = nc.dram_tensor("wk", [D, DG], BF16, kind="ExternalInput").ap()
    wv = nc.dram_tensor("wv", [D, DG], BF16, kind="ExternalInput").ap()
    wo = nc.dram_tensor("wo", [DG, D], BF16, kind="ExternalInput").ap()
    out = nc.dram_tensor("out", [S, D], BF16, kind="ExternalOutput").ap()

    with tile.TileContext(nc) as tc:
        with (
            tc.tile_pool(name="wpool", bufs=1) as wpool,
            tc.tile_pool(name="xkp", bufs=16) as xkp,
            tc.tile_pool(name="xvp", bufs=16) as xvp,
            tc.tile_pool(name="xqp", bufs=2) as xqp,
            tc.tile_pool(name="proj", bufs=1) as proj,
            tc.tile_pool(name="expp", bufs=20) as expp,
            tc.tile_pool(name="outtp", bufs=4) as outtp,
            tc.tile_pool(name="osbp", bufs=1) as osbp,
            tc.tile_pool(name="smallp", bufs=2) as smallp,
        ):
            # ---- weights + persistent activation tiles ---------------------
            wk_sb = [wpool.tile([128, DG], BF16, tag=f"wk{e}", name=f"wk{e}")
                     for e in range(ET)]
            wq_sb = [wpool.tile([128, DG], BF16, tag=f"wq{e}", name=f"wq{e}")
                     for e in range(ET)]
            wv_sb = [wpool.tile([128, DG], BF16, tag=f"wv{e}", name=f"wv{e}")
                     for e in range(ET)]
            wo_sb = [wpool.tile([128, D], BF16, tag=f"wo{p}", name=f"wo{p}")
                     for p in range(2)]

            kt = [proj.tile([128, S], BF16, tag=f"kt{p}", name=f"kt{p}")
                  for p in range(2)]
            qt = [proj.tile([128, S], BF16, tag=f"qt{p}", name=f"qt{p}")
                  for p in range(2)]
            # V in [j, head, dk+1] layout; the 65th column of ones makes the
            # AV matmul emit softmax denominators in PSUM row 64.
            v_sb = proj.tile([128, JT, HL, DK + 1], BF16, tag="v", name="v_sb")
            nc.vector.memset(v_sb[:, :, :, DK:DK + 1], 1.0)

            # Prime the ACT exp table set during the prologue so the first
            # real exp doesn't eat the ~2.7us ACT_TABLE_LOAD mid-pipeline.
            prime = wpool.tile([1, 8], F32, tag="prime", name="prime")
            nc.vector.memset(prime, 0.0)
            nc.scalar.activation(out=prime, in_=prime, func=EXP, scale=1.0)

            QS = (nc.sync, nc.scalar, nc.gpsimd)

            # ---- DMA emission ----------------------------------------------
            # Ordered by when compute needs each ~1MB unit: wk, xk-cc0, wq,
            # xq-c0, xk-cc1..3, xq-c1, wv, xv-cc0..3, wo, xq-c2, xq-c3.
            xk_t = {}
            xv_t = {}
            xq_slab = [None] * QC
            rr = 0

            def rrq():
                nonlocal rr
                rr += 1
                return QS[rr % 3]

            def load_x_cc(dram, tiles, pool, tag, cc):
                for e in range(ET):
                    t = pool.tile([128, 512], BF16, tag=tag,
                                  name=f"{tag}{e}_{cc}")
                    rrq().dma_start(
                        t, dram[e * 128:(e + 1) * 128,
                                cc * 512:(cc + 1) * 512])
                    tiles[(e, cc)] = t

            def load_xq(c, q=None):
                xq_slab[c] = xqp.tile([128, ET, 512], BF16, tag="xqc",
                                      name=f"xq{c}")
                for e in range(ET):
                    eng = QS[q] if q is not None else rrq()
                    eng.dma_start(
                        xq_slab[c][:, e, :],
                        xq[e * 128:(e + 1) * 128, c * 512:(c + 1) * 512])

            for e in range(ET):
                rrq().dma_start(wk_sb[e], wk[e * 128:(e + 1) * 128, :])
            load_x_cc(xk, xk_t, xkp, "xk", 0)
            for e in range(ET):
                rrq().dma_start(wq_sb[e], wq[e * 128:(e + 1) * 128, :])
            load_xq(0)
            for cc in range(1, 4):
                load_x_cc(xk, xk_t, xkp, "xk", cc)
            load_xq(1)
            for e in range(ET):
                rrq().dma_start(wv_sb[e], wv[e * 128:(e + 1) * 128, :])
            for cc in range(4):
                load_x_cc(xv, xv_t, xvp, "xv", cc)
            for p in range(2):
                rrq().dma_start(wo_sb[p], wo[p * 128:(p + 1) * 128, :])

            # ---- phase B ---------------------------------------------------
            ex_tiles = {}
            av_state = {}
            outt = {}
            osb = {}

            with (
                tc.tile_pool(name="ps_sc", bufs=2, space="PSUM") as ps_sc,
                tc.tile_pool(name="ps_av", bufs=2, space="PSUM") as ps_av,
                tc.tile_pool(name="ps_fl", bufs=2, space="PSUM") as ps_fl,
            ):
                def emit_sc_jt(p, c, jt):
                    sc_t = ps_sc.tile([128, 2, 512], F32, tag="sc",
                                      name=f"sc{p}_{c}_{jt}")
                    csl = slice(c * 512, (c + 1) * 512)
                    jsl = slice(jt * 128, (jt + 1) * 128)
                    nc.tensor.matmul(sc_t[:, 0, :], kt[p][0:64, jsl],
                                     qt[p][0:64, csl], start=True, stop=True,
                                     tile_position=(0, 0))
                    nc.tensor.matmul(sc_t[:, 1, :], kt[p][64:128, jsl],
                                     qt[p][64:128, csl], start=True, stop=True,
                                     tile_position=(64, 0))
                    ex = expp.tile([128, 2, 512], BF16, tag="ex",
                                   name=f"ex{p}_{c}_{jt}")
                    nc.scalar.activation(out=ex, in_=sc_t, func=EXP,
                                         scale=SCALE)
                    ex_tiles[(p, c)].append(ex)

                def emit_av_jt(p, c, jt):
                    if jt == 0:
                        av_state[(p, c)] = (
                            ps_av.tile([128, 512], F32, tag="av",
                                       name=f"avA{p}_{c}"),
                            ps_av.tile([128, 512], F32, tag="av",
                                       name=f"avB{p}_{c}"),
                        )
                    avA, avB = av_state[(p, c)]
                    ex = ex_tiles[(p, c)][jt]
                    nc.tensor.matmul(avA[0:DK + 1, :], v_sb[:, jt, 2 * p, :],
                                     ex[:, 0, :],
                                     start=(jt == 0), stop=(jt == JT - 1))
                    nc.tensor.matmul(avB[0:DK + 1, :], v_sb[:, jt, 2 * p + 1, :],
                                     ex[:, 1, :],
                                     start=(jt == 0), stop=(jt == JT - 1))

                def emit_normalize(p, c):
                    # Evacuate both av banks immediately (rs = rowsum row,
                    # st = unnormalized outT), then the reciprocal chain
                    # (DMA-folded to [4,128] so the DVE reciprocal runs
                    # 128-wide) entirely off the PE critical path.
                    avA, avB = av_state[(p, c)]
                    ot = outtp.tile([128, 512], BF16, tag="ot",
                                    name=f"ot{p}_{c}")
                    rss, sts = [], []
                    for i, av_ in ((0, avA), (1, avB)):
                        rs = smallp.tile([1, 512], F32, tag="rs", bufs=2,
                                         name=f"rs{p}_{c}_{i}")
                        nc.vector.tensor_copy(rs, av_[DK:DK + 1, :])
                        st = smallp.tile([64, 512], F32, tag="st", bufs=2,
                                         name=f"st{p}_{c}_{i}")
                        nc.vector.tensor_copy(st, av_[0:DK, :])
                        rss.append(rs)
                        sts.append(st)
                    for i in range(2):
                        rs4 = smallp.tile([4, 128], F32, tag="rs4", bufs=2,
                                          name=f"rs4{p}_{c}_{i}")
                        nc.sync.dma_start(
                            rs4, rss[i].rearrange("one (pp f) -> one pp f",
                                                  pp=4))
                        nc.vector.reciprocal(rs4, rs4)
                        rsu = smallp.tile([1, 512], F32, tag="rsu", bufs=2,
                                          name=f"rsu{p}_{c}_{i}")
                        nc.sync.dma_start(
                            rsu.rearrange("one (pp f) -> one pp f", pp=4),
                            rs4)
                        rbc = smallp.tile([64, 512], F32, tag="rbc", bufs=2,
                                          name=f"rbc{p}_{c}_{i}")
                        nc.gpsimd.partition_broadcast(rbc, rsu)
                        nc.vector.tensor_tensor(
                            ot[i * 64:(i + 1) * 64, :], sts[i], rbc, MULT)
                    outt[(p, c)] = ot

                def emit_kproj(cc):
                    ka = [ps_fl.tile([128, 512], F32, tag="fl",
                                     name=f"ka{cc}_{p}") for p in range(2)]
                    for e in range(ET):
                        for p in range(2):
                            nc.tensor.matmul(
                                ka[p], wk_sb[e][:, p * 128:(p + 1) * 128],
                                xk_t[(e, cc)],
                                start=(e == 0), stop=(e == ET - 1))
                    for p in range(2):
                        nc.vector.tensor_copy(
                            kt[p][:, cc * 512:(cc + 1) * 512], ka[p])

                def emit_qproj(c):
                    qa = [ps_fl.tile([128, 512], F32, tag="fl",
                                     name=f"qa{c}_{p}") for p in range(2)]
                    for e in range(ET):
                        for p in range(2):
                            nc.tensor.matmul(
                                qa[p], wq_sb[e][:, p * 128:(p + 1) * 128],
                                xq_slab[c][:, e, :],
                                start=(e == 0), stop=(e == ET - 1))
                    for p in range(2):
                        nc.vector.tensor_copy(
                            qt[p][:, c * 512:(c + 1) * 512], qa[p])

                def emit_vq(i):
                    # quarter i: j-tiles 4i..4i+3 of V (xv column-chunk cc=i),
                    # projected [j, dg] with the x-tile as stationary operand.
                    for jj in range(4):
                        jt = 4 * i + jj
                        vt = ps_fl.tile([128, 512], F32, tag="fl",
                                        name=f"vq{i}_{jj}")
                        for e in range(ET):
                            nc.tensor.matmul(
                                vt[:, 0:DG],
                                xv_t[(e, i)][:, jj * 128:(jj + 1) * 128],
                                wv_sb[e],
                                start=(e == 0), stop=(e == ET - 1))
                        nc.vector.tensor_copy(
                            v_sb[:, jt, :, 0:DK],
                            vt[:, 0:DG].rearrange("p (h d) -> p h d", h=HL))

                def emit_wo_half(c, half):
                    if half == 0:
                        osb[c] = osbp.tile([128, 4, D], BF16, tag="osb",
                                           bufs=1, name=f"osb{c}")
                    ot0, ot1 = outt[(0, c)], outt[(1, c)]
                    for t4 in (2 * half, 2 * half + 1):
                        for ch in range(2):
                            wp = ps_fl.tile([128, 512], F32, tag="fl",
                                            name=f"wo{c}_{t4}_{ch}")
                            nc.tensor.matmul(
                                wp, ot0[:, t4 * 128:(t4 + 1) * 128],
                                wo_sb[0][:, ch * 512:(ch + 1) * 512],
                                start=True, stop=False)
                            nc.tensor.matmul(
                                wp, ot1[:, t4 * 128:(t4 + 1) * 128],
                                wo_sb[1][:, ch * 512:(ch + 1) * 512],
                                start=False, stop=True)
                            nc.vector.tensor_copy(
                                osb[c][:, t4, ch * 512:(ch + 1) * 512], wp)
                    dst = out[c * 512 + half * 256:
                              c * 512 + (half + 1) * 256, :].rearrange(
                        "(a j) e -> j a e", a=2)
                    nc.sync.dma_start(dst, osb[c][:, 2 * half:2 * half + 2, :])

                def emit_filler(f):
                    if f[0] == "q":
                        emit_qproj(f[1])
                    elif f[0] == "k":
                        emit_kproj(f[1])
                    elif f[0] == "v":
                        emit_vq(f[1])
                    else:
                        emit_wo_half(f[1], f[2])

                # ---- prologue: K chunk 0 + Q chunk 0 (DMA-gated) -----------
                emit_kproj(0)
                emit_qproj(0)

                SLOTS = [(p, c) for c in range(QC) for p in range(2)]
                # (filler, jt-position) per slot. wo fillers sit mid-slot so
                # the previous chunk's reciprocal chain has finished.
                FILLERS = {
                    0: [("k", 1, 2), ("k", 2, 6), ("k", 3, 10), ("q", 1, 13)],
                    1: [("v", 0, 2), ("v", 1, 6), ("v", 2, 10), ("v", 3, 13)],
                    2: [("q", 2, 2)],
                    3: [("wo", 0, 0, 7), ("wo", 0, 1, 12)],
                    4: [("q", 3, 2)],
                    5: [("wo", 1, 0, 7)],
                    6: [("wo", 1, 1, 7)],
                    7: [("wo", 2, 0, 7), ("wo", 2, 1, 12)],
                }
                # AV interleave shift per slot: slot 1 consumes V quarters as
                # they are produced, so its AV trails by 4 j-tiles.
                AV_SHIFT = {1: 4}

                for s in range(9):
                    cur = SLOTS[s] if s < 8 else None
                    prev = SLOTS[s - 1] if s > 0 else None
                    shift = AV_SHIFT.get(s, 2)
                    fillers = list(FILLERS.get(s, []))
                    if cur is not None:
                        ex_tiles[cur] = []
                    if s == 1:
                        load_xq(2, q=0)
                    if s == 3:
                        load_xq(3, q=0)
                    for jt in range(JT):
                        if cur is not None:
                            emit_sc_jt(*cur, jt)
                        while fillers and fillers[0][-1] == jt:
                            emit_filler(fillers.pop(0)[:-1])
                        if prev is not None and jt >= shift:
                            emit_av_jt(*prev, jt - shift)
                    for f in fillers:
                        emit_filler(f[:-1])
                    if prev is not None:
                        for jt in range(JT - shift, JT):
                            emit_av_jt(*prev, jt)
                        emit_normalize(*prev)

                emit_wo_half(3, 0)
                emit_wo_half(3, 1)

    nc.compile()
    return nc


_NC = None


def _get_nc():
    global _NC
    if _NC is None:
        _NC = _build_nc()
    return _NC


def make_in_maps(query, key, value, Wq, Wk, Wv, Wo):
    query = np.ascontiguousarray(query, dtype=np.float32)
    key_ = np.ascontiguousarray(key, dtype=np.float32)
    value = np.ascontiguousarray(value, dtype=np.float32)
    xqT = [np.ascontiguousarray(query[b].T).astype(NPBF) for b in range(B)]
    xkT = [np.ascontiguousarray(key_[b].T).astype(NPBF) for b in range(B)]
    xvT = [np.ascontiguousarray(value[b].T).astype(NPBF) for b in range(B)]
    Wq = np.asarray(Wq, dtype=np.float32)
    Wk = np.asarray(Wk, dtype=np.float32)
    Wv = np.asarray(Wv, dtype=np.float32)
    Wo = np.asarray(Wo, dtype=np.float32)

    in_maps = []
    for core in range(8):
        b, g = divmod(core, 4)
        sl = slice(g * DG, (g + 1) * DG)
        in_maps.append({
            "xq": xqT[b],
            "xk": xkT[b],
            "xv": xvT[b],
            "wq": np.ascontiguousarray(Wq[:, sl]).astype(NPBF),
            "wk": np.ascontiguousarray(Wk[:, sl]).astype(NPBF),
            "wv": np.ascontiguousarray(Wv[:, sl]).astype(NPBF),
            "wo": np.ascontiguousarray(Wo[sl, :]).astype(NPBF),
        })
    return in_maps


def combine_results(results):
    out = np.zeros((B, S, D), dtype=np.float32)
    for core in range(8):
        out[core // 4] += np.asarray(results[core]["out"]).astype(np.float32)
    return out


def kernel(query, key, value, Wq, Wk, Wv, Wo, _trace=False):
    from concourse import bass_utils

    nc = _get_nc()
    in_maps = make_in_maps(query, key, value, Wq, Wk, Wv, Wo)
    r = bass_utils.run_bass_kernel_spmd(
        nc, in_maps, core_ids=list(range(8)), trace=_trace
    )
    kernel.last_results = r
    return combine_results(r.results)


# revision 22
# speedup vs baseline: 1.0328x; 1.0328x over previous
"""Trainium2 Bass kernel for 16-head MultiHeadAttention (B=2, S=2048, D=1024).

Sharding: 8 cores = 2 (batch) x 4 (head groups of 4 heads).
Each core gets a col-shard of Wq/Wk/Wv ([1024,256]) + row-shard of Wo
([256,1024]) and emits ONE full [2048,1024] bf16 partial (K=256 PSUM
accumulation over the two head-pairs); the host sums 4 partials per batch.

All SBUF tensors are bf16 (PSUM accumulation stays f32); inputs arrive
bf16 from the host, halving DMA. Pipeline error ~6e-3 (gate 2e-2).

Pipeline (v3): phase B starts at ~18us because ALL projection work is
chunked into ~1MB-gated filler units that stream behind the first
score matmuls:
 - K/Q projections are chunked by 512-column groups (kproj/qproj units);
   only kproj(0)+qproj(0) (2.5MB of DMA) gate the first scores.
 - V is projected directly in [j, dg] layout (x-tile as the stationary
   operand -- no PE transposes); quarter i needs only xv's column-slice
   cc=i (1MB), so the quarters stream as slot-1 fillers.
 - 8 slots of (head-pair p, q-chunk c): score MM pairs (row-packed via
   tile_position, measured concurrent) interleave 1:1 with the previous
   slot's AV matmuls at j-tile granularity, paced by the ACT engine's
   exp (~1.09us per j-tile, the true bottleneck at ~139us).
 - AV rowsums via the ones-column of V_aug (M=65); the reciprocal runs
   128-wide after a [1,512]->[4,128] DMA fold, off the PE critical path
   (PSUM av banks are evacuated by rs/st copies immediately).
 - PSUM: sc 2x[128,2,512] (4 banks) + av 2 + filler 2 = 8 banks exactly.
"""

import sys

import numpy as np

if "/opt/trn_rl_repo" not in sys.path:
    sys.path.insert(0, "/opt/trn_rl_repo")

import ml_dtypes

import concourse.bacc as bacc
import concourse.mybir as mybir
import concourse.tile as tile

F32 = mybir.dt.float32
BF16 = mybir.dt.bfloat16
NPBF = ml_dtypes.bfloat16

B, S, D, H = 2, 2048, 1024, 16
DK = D // H          # 64
HL = 4               # heads per core
DG = HL * DK         # 256
SCALE = 0.125        # 1/sqrt(DK)

ET = D // 128        # 8 e-tiles (contraction tiles for projections)
JT = S // 128        # 16 j-tiles (key positions)
QC = 4               # q-chunks of 512 per head-pair
EXP = mybir.ActivationFunctionType.Exp
MULT = mybir.AluOpType.mult


def _build_nc():
    nc = bacc.Bacc("TRN2", target_bir_lowering=False, debug=False)

    xq = nc.dram_tensor("xq", [D, S], BF16, kind="ExternalInput").ap()
    xk = nc.dram_tensor("xk", [D, S], BF16, kind="ExternalInput").ap()
    xv = nc.dram_tensor("xv", [D, S], BF16, kind="ExternalInput").ap()
    wq = nc.dram_tensor("wq", [D, DG], BF16, kind="ExternalInput").ap()
    wk = nc.dram_tensor("wk", [D, DG], BF16, kind="ExternalInput").ap()
    wv = nc.dram_tensor("wv", [D, DG], BF16, kind="ExternalInput").ap()
    wo = nc.dram_tensor("wo", [DG, D], BF16, kind="ExternalInput").ap()
    out = nc.dram_tensor("out", [S, D], BF16, kind="ExternalOutput").ap()

    with tile.TileContext(nc) as tc:
        with (
            tc.tile_pool(name="wpool", bufs=1) as wpool,
            tc.tile_pool(name="xkp", bufs=2) as xkp,
            tc.tile_pool(name="xvp", bufs=4) as xvp,
            tc.tile_pool(name="xqp", bufs=2) as xqp,
            tc.tile_pool(name="proj", bufs=1) as proj,
            tc.tile_pool(name="expp", bufs=19) as expp,
            tc.tile_pool(name="outtp", bufs=4) as outtp,
            tc.tile_pool(name="osbp", bufs=1) as osbp,
            tc.tile_pool(name="smallp", bufs=2) as smallp,
        ):
            # ---- weights + persistent activation tiles ---------------------
            # Weight slabs: [128, e, DG] so each full weight loads with ONE
            # dma_start (small transfers + many ring pushes were the v3
            # bottleneck: scalar-queue pushes blocked the ACT sequencer and
            # 128KB transfers ran far below line rate).
            wk_sl = wpool.tile([128, ET, DG], BF16, tag="wk", name="wk_sl")
            wq_sl = wpool.tile([128, ET, DG], BF16, tag="wq", name="wq_sl")
            wv_sl = wpool.tile([128, ET, DG], BF16, tag="wv", name="wv_sl")
            wk_sb = [wk_sl[:, e, :] for e in range(ET)]
            wq_sb = [wq_sl[:, e, :] for e in range(ET)]
            wv_sb = [wv_sl[:, e, :] for e in range(ET)]
            wo_sb = [wpool.tile([128, D], BF16, tag=f"wo{p}", name=f"wo{p}")
                     for p in range(2)]

            kt = [proj.tile([128, S], BF16, tag=f"kt{p}", name=f"kt{p}")
                  for p in range(2)]
            qt = [proj.tile([128, S], BF16, tag=f"qt{p}", name=f"qt{p}")
                  for p in range(2)]
            # V in [j, head, dk+1] layout; the 65th column of ones makes the
            # AV matmul emit softmax denominators in PSUM row 64.
            v_sb = proj.tile([128, JT, HL, DK + 1], BF16, tag="v", name="v_sb")
            nc.vector.memset(v_sb[:, :, :, DK:DK + 1], 1.0)

            # Prime the ACT exp table set during the prologue so the first
            # real exp doesn't eat the ~2.7us ACT_TABLE_LOAD mid-pipeline.
            prime = wpool.tile([1, 8], F32, tag="prime", name="prime")
            nc.vector.memset(prime, 0.0)
            nc.scalar.activation(out=prime, in_=prime, func=EXP, scale=1.0)

            QS = (nc.sync, nc.scalar, nc.gpsimd)

            # ---- DMA emission ----------------------------------------------
            # One big dma_start per ~0.5-1MB unit, on the sync + gpsimd
            # HW queues only (NEVER the scalar queue: its ring pushes stall
            # the ACT sequencer and with it the whole exp pipeline). Each
            # ring is need-ordered.
            xk_slab = {}
            xv_slab = {}
            xq_slab = [None] * QC

            def x_src(dram, cc):
                return dram[:, cc * 512:(cc + 1) * 512].rearrange(
                    "(e p) f -> p e f", e=ET)

            def load_x_cc(dram, slabs, pool, tag, cc, q):
                slab = pool.tile([128, ET, 512], BF16, tag=tag,
                                 name=f"{tag}_cc{cc}")
                QS[q].dma_start(slab, x_src(dram, cc))
                slabs[cc] = slab

            def load_xq(c, q):
                xq_slab[c] = xqp.tile([128, ET, 512], BF16, tag="xqc",
                                      name=f"xq{c}")
                QS[q].dma_start(xq_slab[c], x_src(xq, c))

            w_src = lambda w: w.rearrange("(e p) f -> p e f", e=ET)
            # sync ring: what the first scores need, then fill-ins.
            nc.sync.dma_start(wk_sl, w_src(wk))
            load_x_cc(xk, xk_slab, xkp, "xk", 0, 0)
            nc.sync.dma_start(wq_sl, w_src(wq))
            load_xq(0, 0)
            load_xq(1, 0)
            load_x_cc(xv, xv_slab, xvp, "xv", 1, 0)
            load_x_cc(xv, xv_slab, xvp, "xv", 3, 0)
            # gpsimd ring: remaining K chunks, V path, Wo.
            load_x_cc(xk, xk_slab, xkp, "xk", 1, 2)
            load_x_cc(xk, xk_slab, xkp, "xk", 2, 2)
            load_x_cc(xk, xk_slab, xkp, "xk", 3, 2)
            nc.gpsimd.dma_start(wv_sl, w_src(wv))
            load_x_cc(xv, xv_slab, xvp, "xv", 0, 2)
            load_x_cc(xv, xv_slab, xvp, "xv", 2, 2)
            for p in range(2):
                nc.gpsimd.dma_start(wo_sb[p], wo[p * 128:(p + 1) * 128, :])

            # ---- phase B ---------------------------------------------------
            ex_tiles = {}
            av_state = {}
            outt = {}
            osb = {}

            with (
                tc.tile_pool(name="ps_sc", bufs=2, space="PSUM") as ps_sc,
                tc.tile_pool(name="ps_av", bufs=2, space="PSUM") as ps_av,
                tc.tile_pool(name="ps_fl", bufs=2, space="PSUM") as ps_fl,
            ):
                def emit_sc_jt(p, c, jt):
                    sc_t = ps_sc.tile([128, 2, 512], F32, tag="sc",
                                      name=f"sc{p}_{c}_{jt}")
                    csl = slice(c * 512, (c + 1) * 512)
                    jsl = slice(jt * 128, (jt + 1) * 128)
                    nc.tensor.matmul(sc_t[:, 0, :], kt[p][0:64, jsl],
                                     qt[p][0:64, csl], start=True, stop=True,
                                     tile_position=(0, 0))
                    nc.tensor.matmul(sc_t[:, 1, :], kt[p][64:128, jsl],
                                     qt[p][64:128, csl], start=True, stop=True,
                                     tile_position=(64, 0))
                    ex = expp.tile([128, 2, 512], BF16, tag="ex",
                                   name=f"ex{p}_{c}_{jt}")
                    nc.scalar.activation(out=ex, in_=sc_t, func=EXP,
                                         scale=SCALE)
                    ex_tiles[(p, c)].append(ex)

                def emit_av_jt(p, c, jt):
                    if jt == 0:
                        av_state[(p, c)] = (
                            ps_av.tile([128, 512], F32, tag="av",
                                       name=f"avA{p}_{c}"),
                            ps_av.tile([128, 512], F32, tag="av",
                                       name=f"avB{p}_{c}"),
                        )
                    avA, avB = av_state[(p, c)]
                    ex = ex_tiles[(p, c)][jt]
                    nc.tensor.matmul(avA[0:DK + 1, :], v_sb[:, jt, 2 * p, :],
                                     ex[:, 0, :],
                                     start=(jt == 0), stop=(jt == JT - 1))
                    nc.tensor.matmul(avB[0:DK + 1, :], v_sb[:, jt, 2 * p + 1, :],
                                     ex[:, 1, :],
                                     start=(jt == 0), stop=(jt == JT - 1))

                def emit_normalize(p, c):
                    # Evacuate both av banks immediately (rs = rowsum row,
                    # st = unnormalized outT), then the reciprocal chain
                    # (DMA-folded to [4,128] so the DVE reciprocal runs
                    # 128-wide) entirely off the PE critical path.
                    avA, avB = av_state[(p, c)]
                    ot = outtp.tile([128, 512], BF16, tag="ot",
                                    name=f"ot{p}_{c}")
                    rss, sts = [], []
                    for i, av_ in ((0, avA), (1, avB)):
                        rs = smallp.tile([1, 512], F32, tag="rs", bufs=2,
                                         name=f"rs{p}_{c}_{i}")
                        nc.vector.tensor_copy(rs, av_[DK:DK + 1, :])
                        st = smallp.tile([64, 512], F32, tag="st", bufs=2,
                                         name=f"st{p}_{c}_{i}")
                        nc.vector.tensor_copy(st, av_[0:DK, :])
                        rss.append(rs)
                        sts.append(st)
                    for i in range(2):
                        rs4 = smallp.tile([4, 128], F32, tag="rs4", bufs=2,
                                          name=f"rs4{p}_{c}_{i}")
                        nc.sync.dma_start(
                            rs4, rss[i].rearrange("one (pp f) -> one pp f",
                                                  pp=4))
                        nc.vector.reciprocal(rs4, rs4)
                        rsu = smallp.tile([1, 512], F32, tag="rsu", bufs=2,
                                          name=f"rsu{p}_{c}_{i}")
                        nc.sync.dma_start(
                            rsu.rearrange("one (pp f) -> one pp f", pp=4),
                            rs4)
                        rbc = smallp.tile([64, 512], F32, tag="rbc", bufs=2,
                                          name=f"rbc{p}_{c}_{i}")
                        nc.gpsimd.partition_broadcast(rbc, rsu)
                        nc.vector.tensor_tensor(
                            ot[i * 64:(i + 1) * 64, :], sts[i], rbc, MULT)
                    outt[(p, c)] = ot

                def emit_kproj(cc):
                    ka = [ps_fl.tile([128, 512], F32, tag="fl",
                                     name=f"ka{cc}_{p}") for p in range(2)]
                    for e in range(ET):
                        for p in range(2):
                            nc.tensor.matmul(
                                ka[p], wk_sb[e][:, p * 128:(p + 1) * 128],
                                xk_slab[cc][:, e, :],
                                start=(e == 0), stop=(e == ET - 1))
                    for p in range(2):
                        nc.vector.tensor_copy(
                            kt[p][:, cc * 512:(cc + 1) * 512], ka[p])

                def emit_qproj(c):
                    qa = [ps_fl.tile([128, 512], F32, tag="fl",
                                     name=f"qa{c}_{p}") for p in range(2)]
                    for e in range(ET):
                        for p in range(2):
                            nc.tensor.matmul(
                                qa[p], wq_sb[e][:, p * 128:(p + 1) * 128],
                                xq_slab[c][:, e, :],
                                start=(e == 0), stop=(e == ET - 1))
                    for p in range(2):
                        nc.vector.tensor_copy(
                            qt[p][:, c * 512:(c + 1) * 512], qa[p])

                def emit_vq(i):
                    # quarter i: j-tiles 4i..4i+3 of V (xv column-chunk cc=i),
                    # projected [j, dg] with the x-tile as stationary operand.
                    for jj in range(4):
                        jt = 4 * i + jj
                        vt = ps_fl.tile([128, 512], F32, tag="fl",
                                        name=f"vq{i}_{jj}")
                        for e in range(ET):
                            nc.tensor.matmul(
                                vt[:, 0:DG],
                                xv_slab[i][:, e, jj * 128:(jj + 1) * 128],
                                wv_sb[e],
                                start=(e == 0), stop=(e == ET - 1))
                        nc.vector.tensor_copy(
                            v_sb[:, jt, :, 0:DK],
                            vt[:, 0:DG].rearrange("p (h d) -> p h d", h=HL))

                def emit_wo_half(c, half):
                    if half == 0:
                        osb[c] = osbp.tile([128, 4, D], BF16, tag="osb",
                                           bufs=1, name=f"osb{c}")
                    ot0, ot1 = outt[(0, c)], outt[(1, c)]
                    for t4 in (2 * half, 2 * half + 1):
                        for ch in range(2):
                            wp = ps_fl.tile([128, 512], F32, tag="fl",
                                            name=f"wo{c}_{t4}_{ch}")
                            nc.tensor.matmul(
                                wp, ot0[:, t4 * 128:(t4 + 1) * 128],
                                wo_sb[0][:, ch * 512:(ch + 1) * 512],
                                start=True, stop=False)
                            nc.tensor.matmul(
                                wp, ot1[:, t4 * 128:(t4 + 1) * 128],
                                wo_sb[1][:, ch * 512:(ch + 1) * 512],
                                start=False, stop=True)
                            nc.vector.tensor_copy(
                                osb[c][:, t4, ch * 512:(ch + 1) * 512], wp)
                    dst = out[c * 512 + half * 256:
                              c * 512 + (half + 1) * 256, :].rearrange(
                        "(a j) e -> j a e", a=2)
                    nc.sync.dma_start(dst, osb[c][:, 2 * half:2 * half + 2, :])

                def emit_filler(f):
                    if f[0] == "q":
                        emit_qproj(f[1])
                    elif f[0] == "k":
                        emit_kproj(f[1])
                    elif f[0] == "v":
                        emit_vq(f[1])
                    else:
                        emit_wo_half(f[1], f[2])

                # ---- prologue: K chunk 0 + Q chunk 0 (DMA-gated) -----------
                emit_kproj(0)
                emit_qproj(0)

                SLOTS = [(p, c) for c in range(QC) for p in range(2)]
                # (filler, jt-position) per slot. wo fillers sit mid-slot so
                # the previous chunk's reciprocal chain has finished.
                FILLERS = {
                    0: [("k", 1, 2), ("k", 2, 6), ("k", 3, 10), ("q", 1, 13)],
                    1: [("v", 0, 2), ("v", 1, 6), ("v", 2, 10), ("v", 3, 13)],
                    2: [("q", 2, 2)],
                    3: [("wo", 0, 0, 7), ("wo", 0, 1, 12)],
                    4: [("q", 3, 2)],
                    5: [("wo", 1, 0, 7)],
                    6: [("wo", 1, 1, 7)],
                    7: [("wo", 2, 0, 7), ("wo", 2, 1, 12)],
                }
                # AV interleave shift per slot: slot 1 consumes V quarters as
                # they are produced, so its AV trails by 4 j-tiles.
                AV_SHIFT = {1: 4}

                for s in range(9):
                    cur = SLOTS[s] if s < 8 else None
                    prev = SLOTS[s - 1] if s > 0 else None
                    shift = AV_SHIFT.get(s, 2)
                    fillers = list(FILLERS.get(s, []))
                    if cur is not None:
                        ex_tiles[cur] = []
                    if s == 1:
                        load_xq(2, 0)
                    if s == 3:
                        load_xq(3, 0)
                    for jt in range(JT):
                        if cur is not None:
                            emit_sc_jt(*cur, jt)
                        while fillers and fillers[0][-1] == jt:
                            emit_filler(fillers.pop(0)[:-1])
                        if prev is not None and jt >= shift:
                            emit_av_jt(*prev, jt - shift)
                    for f in fillers:
                        emit_filler(f[:-1])
                    if prev is not None:
                        for jt in range(JT - shift, JT):
                            emit_av_jt(*prev, jt)
                        emit_normalize(*prev)

                emit_wo_half(3, 0)
                emit_wo_half(3, 1)

    nc.compile()
    return nc


_NC = None


def _get_nc():
    global _NC
    if _NC is None:
        _NC = _build_nc()
    return _NC


def make_in_maps(query, key, value, Wq, Wk, Wv, Wo):
    query = np.ascontiguousarray(query, dtype=np.float32)
    key_ = np.ascontiguousarray(key, dtype=np.float32)
    value = np.ascontiguousarray(value, dtype=np.float32)
    xqT = [np.ascontiguousarray(query[b].T).astype(NPBF) for b in range(B)]
    xkT = [np.ascontiguousarray(key_[b].T).astype(NPBF) for b in range(B)]
    xvT = [np.ascontiguousarray(value[b].T).astype(NPBF) for b in range(B)]
    Wq = np.asarray(Wq, dtype=np.float32)
    Wk = np.asarray(Wk, dtype=np.float32)
    Wv = np.asarray(Wv, dtype=np.float32)
    Wo = np.asarray(Wo, dtype=np.float32)

    in_maps = []
    for core in range(8):
        b, g = divmod(core, 4)
        sl = slice(g * DG, (g + 1) * DG)
        in_maps.append({
            "xq": xqT[b],
            "xk": xkT[b],
            "xv": xvT[b],
            "wq": np.ascontiguousarray(Wq[:, sl]).astype(NPBF),
            "wk": np.ascontiguousarray(Wk[:, sl]).astype(NPBF),
            "wv": np.ascontiguousarray(Wv[:, sl]).astype(NPBF),
            "wo": np.ascontiguousarray(Wo[sl, :]).astype(NPBF),
        })
    return in_maps


def combine_results(results):
    out = np.zeros((B, S, D), dtype=np.float32)
    for core in range(8):
        out[core // 4] += np.asarray(results[core]["out"]).astype(np.float32)
    return out


def kernel(query, key, value, Wq, Wk, Wv, Wo, _trace=False):
    from concourse import bass_utils

    nc = _get_nc()
    in_maps = make_in_maps(query, key, value, Wq, Wk, Wv, Wo)
    r = bass_utils.run_bass_kernel_spmd(
        nc, in_maps, core_ids=list(range(8)), trace=_trace
    )
    kernel.last_results = r
    return combine_results(r.results)
